# revision 18
# baseline (speedup 1.0000x reference)
"""Trainium2 Bass kernel for capsule dynamic routing (nn_Capsule).

Reference (per batch item b):
    u = x_b @ W; logits = 0
    for i in 4:
        c = softmax(logits, axis=capsule)
        t_j = sum_s c[s,j] * u[s, j*64:(j+1)*64]; v = squash(t)
        if i < 3: logits[s,j] += u[s, jblk] . v_j

Never materializes u. By linearity:
    y_q   = sum_s c[s,q] x_s        (GEMM over S, X natural layout)
    T     = y @ W, t = blockdiag(T) (dense q=(b,j) partition layout)
    P     = W^T' Vblk               (Vblk = block-diag of v)
    upd   = X P                     (GEMM over H, X^T layout)

Sharding: batch-parallel, 8 batch items per core, W replicated.
Dense partition index q = b*16 + j (8 batch x 16 capsules = 128).

Perf design (vs. the first working version):
  - All big operands are pre-packed on HOST into the exact f16 SBUF
    layouts (x, x^T, W, W^T) -> contiguous HWDGE DMA loads, no SWDGE
    casts, no element-strided on-chip transpose DMAs.
  - T and P are computed in the dense q layout: 16 matmuls of
    N=512/128 per step instead of 64 padded-256 matmuls.
  - squash runs in natural layout (q on partitions, d on free axis):
    DVE reduce + ACT sqrt, no PE broadcast matmuls.

HW lessons kept from the previous version:
  - ScalarE activation(Copy) for every PSUM f32 -> f16 cast (DVE dies).
  - Each PE-transpose output gets its own PSUM tile.
  - matmul start=True lazily zeroes the PSUM bank for the out AP's
    partitions; partition-disjoint groups interleave with
    skip_group_check=True.
  - f16 constants come from host DRAM, not memset.
"""
import numpy as np
from contextlib import ExitStack

import concourse.bass as bass
import concourse.bacc as bacc
import concourse.tile as tile
from concourse import mybir
from concourse.bass_utils import run_bass_kernel_spmd

f16 = mybir.dt.float16
f32 = mybir.dt.float32
COPY = mybir.ActivationFunctionType.Copy
EXP = mybir.ActivationFunctionType.Exp
LN = mybir.ActivationFunctionType.Ln

S, B, H = 512, 64, 1024
NCAP, DCAP = 16, 64
ROUTINGS = 4
N_CORES = 8
BL = B // N_CORES          # 8 batch items per core
SC = S // 128              # 4 s-chunks
HC = H // 128              # 8 h-chunks
OC = H // 128              # 8 o-chunks (o = NCAP*DCAP = 1024)


def _act_copy(nc, out, in_):
    nc.scalar.activation(out=out, in_=in_, func=COPY, scale=1.0, alpha=0.0)


def _build_kernel(tc, out_d, x_d, xt_d, w_d, wt_d, id16_d, id32_d, eps_d,
                  cpad_d, logits_d, vblk_d, tt_d):
    nc = tc.nc
    ctx = ExitStack()
    const = ctx.enter_context(tc.tile_pool(name="const", bufs=1))
    work = ctx.enter_context(tc.tile_pool(name="work", bufs=2))
    small = ctx.enter_context(tc.tile_pool(name="small", bufs=2))
    # PSUM budget: 8 banks. big: y_ps/T_ps/PT_ps/u_ps cycle one 4-bank
    # slot (bufs=1); tp: all transposes, 4 x 1 bank.
    ps_big = ctx.enter_context(tc.tile_pool(name="ps_big", bufs=1,
                                            space="PSUM"))
    ps_tp = ctx.enter_context(tc.tile_pool(name="ps_tp", bufs=4, space="PSUM"))

    # ---------- persistent tensors ----------
    x16 = const.tile([128, BL, SC, 1024], f16)    # X natural (s_loc, b, sc, h)
    xt16 = const.tile([128, BL, HC, 512], f16)    # X^T (h_loc, b, hc, s)
    w16 = const.tile([128, HC, 1024], f16)        # W natural (h_loc, hc, o)
    wt16 = const.tile([128, OC, 1024], f16)       # W^T (o_loc, oc, h)
    id16 = const.tile([128, 128], f16)            # eye(128) f16
    id32 = const.tile([128, 128], f32)            # eye(128) f32
    eps = const.tile([128, 1], f32)
    cpad = const.tile([128, BL, SC, 32], f16)     # c, cols 16-31 zero pad
    logits = const.tile([128, BL, SC, 16], f32)
    vblk = const.tile([128, OC, 128], f16)        # block-diag v, dense q cols
    tt16 = const.tile([128, 128], f16)            # t^T split, zeros elsewhere

    # ---------- loads (all contiguous, host-prepacked f16) ----------
    # Small consts FIRST so iteration-0 compute can ride the x16 chunks.
    nc.sync.dma_start(out=id16[:], in_=id16_d[:])
    nc.sync.dma_start(out=id32[:], in_=id32_d[:])
    nc.sync.dma_start(out=eps[:], in_=eps_d[:])
    nc.sync.dma_start(out=cpad[:], in_=cpad_d[:])
    nc.sync.dma_start(out=logits[:], in_=logits_d[:])
    nc.sync.dma_start(out=vblk[:], in_=vblk_d[:])
    nc.sync.dma_start(out=tt16[:], in_=tt_d[:])
    for sc in range(SC):
        for b in range(BL):
            nc.sync.dma_start(out=x16[:, b, sc, :], in_=x_d[:, b, sc, :])
    nc.sync.dma_start(out=w16[:], in_=w_d[:])
    nc.sync.dma_start(out=wt16[:], in_=wt_d[:])
    for b in range(BL):
        nc.sync.dma_start(out=xt16[:, b, :, :], in_=xt_d[:, b, :, :])

    v32 = None
    for it in range(ROUTINGS):
        last = it == ROUTINGS - 1

        # ---------- y = C^T X: (2g x 4b x 32pad part, 1024 h) f32 ----------
        y_ps = ps_big.tile([128, 2, 1024], f32, tag="big", name=f"y_ps{it}")
        for g in range(2):
            for half in range(2):
                for sc in range(SC):
                    for b_ in range(4):
                        b = 4 * g + b_
                        nc.tensor.matmul(
                            y_ps[32 * b_:32 * b_ + 32, g,
                                 512 * half:512 * half + 512],
                            cpad[:, b, sc, :],
                            x16[:, b, sc, 512 * half:512 * half + 512],
                            start=(sc == 0), stop=(sc == SC - 1),
                            skip_group_check=True,
                            tile_position=(0, 32 * b_))
        y_sb = work.tile([128, 2, 1024], f16, tag="y_sb")
        for g in range(2):
            for half in range(2):
                _act_copy(nc, y_sb[:, g, 512 * half:512 * half + 512],
                          y_ps[:, g, 512 * half:512 * half + 512])

        # ---------- y^T via PE transposes, dense-packed q cols ----------
        yt = work.tile([128, HC, 128], f16, tag="yt")
        for hc in range(HC):
            for g in range(2):
                tp = ps_tp.tile([128, 128], f16, tag="tp",
                                name=f"yt_tp{it}_{hc}_{g}")
                nc.tensor.matmul(
                    tp[:], y_sb[:, g, 128 * hc:128 * hc + 128], id16[:],
                    is_transpose=True, skip_group_check=True)
                # dense capsule-major pack: yt col q = j*8 + b
                src = tp.rearrange("p (b j) -> p j b", j=32)[:, 0:16, :]
                dst = yt[:, hc, :].rearrange(
                    "p (j b) -> p j b", b=8)[:, :, 4 * g:4 * g + 4]
                if (hc + 2 * g) % 2 == 0:
                    nc.vector.tensor_copy(dst, src)
                else:
                    _act_copy(nc, dst, src)

        # ---------- T = y W: (128 q, 1024 o) f32, dense ----------
        t_ps = ps_big.tile([128, 1024], f32, tag="big", name=f"t_ps{it}")
        for half in range(2):
            for hc in range(HC):
                nc.tensor.matmul(
                    t_ps[:, 512 * half:512 * half + 512],
                    yt[:, hc, :],
                    w16[:, hc, 512 * half:512 * half + 512],
                    start=(hc == 0), stop=(hc == HC - 1))
        t_sb16 = work.tile([128, 1024], f16, tag="t_sb16")
        for half in range(2):
            _act_copy(nc, t_sb16[:, 512 * half:512 * half + 512],
                      t_ps[:, 512 * half:512 * half + 512])

        # ---------- T^T chunks via PE transpose; aligned extracts -------
        # tt16[par*64 + d, q] = t[q, d] for q in [8j, 8j+8), j = 2oc+par;
        # other cells stay zero (host-initialized).
        for oc in range(OC):
            ttp = ps_tp.tile([128, 128], f16, tag="tp",
                             name=f"tt_tp{it}_{oc}")
            nc.tensor.matmul(ttp[:], t_sb16[:, 128 * oc:128 * oc + 128],
                             id16[:], is_transpose=True,
                             skip_group_check=True)
            for par in range(2):
                j = 2 * oc + par
                p0 = 64 * par
                if par == 0:
                    nc.vector.tensor_copy(
                        tt16[p0:p0 + 64, 8 * j:8 * j + 8],
                        ttp[p0:p0 + 64, 8 * j:8 * j + 8])
                else:
                    _act_copy(nc, tt16[p0:p0 + 64, 8 * j:8 * j + 8],
                              ttp[p0:p0 + 64, 8 * j:8 * j + 8])

        # ---------- transpose to t[q, d'] (d' = d + 64*(j%2)) ----------
        ttq = ps_tp.tile([128, 128], f16, tag="tp", name=f"ttq{it}")
        nc.tensor.matmul(ttq[:], tt16[:], id16[:],
                         is_transpose=True, skip_group_check=True)
        t_sb = small.tile([128, 128], f16, tag="t_sb")
        nc.vector.tensor_copy(t_sb[:], ttq[:])

        # ---------- squash: v = t / sqrt(sum_d t^2 + eps) ----------
        t2 = small.tile([128, 128], f32, tag="t2")
        nc.vector.tensor_mul(t2[:], t_sb[:], t_sb[:])
        ssum = small.tile([128, 1], f32, tag="ssum")
        nc.vector.reduce_sum(ssum[:], t2[:], axis=mybir.AxisListType.X)
        lnn = small.tile([128, 1], f32, tag="lnn")
        nc.scalar.activation(out=lnn[:], in_=ssum[:], func=LN,
                             bias=eps[:], scale=1.0, alpha=0.0)
        rs = small.tile([128, 1], f32, tag="rs")
        nc.scalar.activation(out=rs[:], in_=lnn[:], func=EXP,
                             scale=-0.5, alpha=0.0)
        if last:
            v32 = small.tile([128, 128], f32, tag="v32")
            nc.vector.tensor_mul(v32[:], t_sb[:],
                                 rs.broadcast_to([128, 128]))
            break
        v16 = small.tile([128, 128], f16, tag="v16")
        nc.vector.tensor_mul(v16[:], t_sb[:], rs.broadcast_to([128, 128]))

        # ---------- scatter v into block-diag vblk (via transpose) -------
        vtp = ps_tp.tile([128, 128], f16, tag="tp", name=f"v_tp{it}")
        nc.tensor.matmul(vtp[:], v16[:], id16[:],
                         is_transpose=True, skip_group_check=True)
        for oc in range(OC):
            for par in range(2):
                j = 2 * oc + par
                p0, p1 = 64 * par, 64 * par + 64
                if par == 0:
                    nc.vector.tensor_copy(vblk[p0:p1, oc, 8 * j:8 * j + 8],
                                          vtp[p0:p1, 8 * j:8 * j + 8])
                else:
                    _act_copy(nc, vblk[p0:p1, oc, 8 * j:8 * j + 8],
                              vtp[p0:p1, 8 * j:8 * j + 8])

        # ---------- P^T = Vblk^T W^T: (128 q, 1024 h) f32, dense -------
        pt_ps = ps_big.tile([128, 1024], f32, tag="big", name=f"pt_ps{it}")
        for half in range(2):
            for oc in range(OC):
                nc.tensor.matmul(
                    pt_ps[:, 512 * half:512 * half + 512],
                    vblk[:, oc, :],
                    wt16[:, oc, 512 * half:512 * half + 512],
                    start=(oc == 0), stop=(oc == OC - 1))
        pt_sb = work.tile([128, 1024], f16, tag="pt_sb")
        for half in range(2):
            _act_copy(nc, pt_sb[:, 512 * half:512 * half + 512],
                      pt_ps[:, 512 * half:512 * half + 512])

        # ---------- transpose P^T -> P natural, batch-major cols --------
        p_sb = work.tile([128, HC, 128], f16, tag="p_sb")
        for hc in range(HC):
            ptp = ps_tp.tile([128, 128], f16, tag="tp",
                             name=f"p_tp{it}_{hc}")
            nc.tensor.matmul(
                ptp[:], pt_sb[:, 128 * hc:128 * hc + 128], id16[:],
                is_transpose=True, skip_group_check=True)
            # permute cols capsule-major q=j*8+b -> batch-major b*16+j
            nc.vector.tensor_copy(
                p_sb[:, hc, :].rearrange("p (b j) -> p j b", j=16),
                ptp.rearrange("p (j b) -> p j b", b=8))

        # ---------- update = X P via X^T (dense 16-col weights) ----------
        u_ps = ps_big.tile([128, 2, 512], f32, tag="big", name=f"u_ps{it}")
        for g in range(2):
            for hc in range(HC):
                for b_ in range(4):
                    b = 4 * g + b_
                    nc.tensor.matmul(
                        u_ps[32 * b_:32 * b_ + 16, g, :],
                        p_sb[:, hc, 16 * b:16 * b + 16],
                        xt16[:, b, hc, :],
                        start=(hc == 0), stop=(hc == HC - 1),
                        skip_group_check=True,
                        tile_position=(0, 32 * b_))
        u_sb = work.tile([128, 2, 512], f32, tag="u_sb")
        for g in range(2):
            nc.vector.tensor_copy(u_sb[:, g, :], u_ps[:, g, :])

        # ---------- transpose update, accumulate logits ----------
        for sc in range(SC):
            for g in range(2):
                utp = ps_tp.tile([128, 128], f32, tag="tp",
                                 name=f"ut_tp{it}_{sc}_{g}")
                nc.tensor.matmul(
                    utp[:], u_sb[:, g, 128 * sc:128 * sc + 128], id32[:],
                    is_transpose=True, skip_group_check=True)
                src = utp.rearrange("p (b j) -> p b j", j=32)
                nc.vector.tensor_add(
                    logits[:, 4 * g:4 * g + 4, sc, :],
                    logits[:, 4 * g:4 * g + 4, sc, :], src[:, :, 0:16])

        # ---------- softmax over capsules -> cpad ----------
        for sc in range(SC):
            ex = small.tile([128, BL, 16], f32, tag="ex")
            nc.scalar.activation(out=ex[:], in_=logits[:, :, sc, :],
                                 func=EXP, scale=1.0, alpha=0.0)
            sm = small.tile([128, BL, 1], f32, tag="sm")
            nc.vector.reduce_sum(sm[:], ex[:], axis=mybir.AxisListType.X)
            rc = small.tile([128, BL, 1], f32, tag="rc")
            nc.vector.reciprocal(rc[:], sm[:])
            nc.vector.tensor_mul(cpad[:, :, sc, 0:16], ex[:],
                                 rc.broadcast_to([128, BL, 16]))

    # ---------- out[b, j, d] = v32[q = j*8 + b, d + 64*(j%2)] ----------
    for j in range(NCAP):
        out_ap = bass.AP(tensor=out_d.tensor, offset=64 * j,
                         ap=[[1024, 8], [1, 64]])
        c0 = 64 * (j % 2)
        nc.sync.dma_start(out=out_ap,
                          in_=v32[8 * j:8 * j + 8, c0:c0 + 64])
    ctx.close()


_CACHE = {}


def _host_consts():
    ident = np.ascontiguousarray(np.eye(128, dtype=np.float16))
    ident32 = np.ascontiguousarray(np.eye(128, dtype=np.float32))
    eps = np.full((128, 1), 1e-7, np.float32)
    cpad = np.zeros((128, BL, SC, 32), np.float16)
    cpad[:, :, :, 0:16] = 1.0 / NCAP          # iteration-0 softmax is exact
    logi = np.zeros((128, BL, SC, 16), np.float32)
    vblk = np.zeros((128, OC, 128), np.float16)
    tti = np.zeros((128, 128), np.float16)
    return {"id16": ident, "id32": ident32, "epsb": eps,
            "cpadi": cpad, "logi": logi, "vblki": vblk, "tti": tti}


def _get_nc():
    if "nc" not in _CACHE:
        nc = bacc.Bacc("TRN2", target_bir_lowering=False, debug=False)
        x_d = nc.dram_tensor("xh", [128, BL, SC, 1024], f16,
                             kind="ExternalInput")
        xt_d = nc.dram_tensor("xth", [128, BL, HC, 512], f16,
                              kind="ExternalInput")
        w_d = nc.dram_tensor("wh", [128, HC, 1024], f16,
                             kind="ExternalInput")
        wt_d = nc.dram_tensor("wth", [128, OC, 1024], f16,
                              kind="ExternalInput")
        id16_d = nc.dram_tensor("id16", [128, 128], f16,
                                kind="ExternalInput")
        id32_d = nc.dram_tensor("id32", [128, 128], f32,
                                kind="ExternalInput")
        eps_d = nc.dram_tensor("epsb", [128, 1], f32, kind="ExternalInput")
        cpad_d = nc.dram_tensor("cpadi", [128, BL, SC, 32], f16,
                                kind="ExternalInput")
        logits_d = nc.dram_tensor("logi", [128, BL, SC, 16], f32,
                                  kind="ExternalInput")
        vblk_d = nc.dram_tensor("vblki", [128, OC, 128], f16,
                                kind="ExternalInput")
        tt_d = nc.dram_tensor("tti", [128, 128], f16, kind="ExternalInput")
        out_d = nc.dram_tensor("out", [BL, NCAP, DCAP], f32,
                               kind="ExternalOutput")
        with tile.TileContext(nc) as tc:
            _build_kernel(tc, out_d.ap(), x_d.ap(), xt_d.ap(), w_d.ap(),
                          wt_d.ap(), id16_d.ap(), id32_d.ap(), eps_d.ap(),
                          cpad_d.ap(), logits_d.ap(), vblk_d.ap(), tt_d.ap())
        nc.compile()
        _CACHE["nc"] = nc
    return _CACHE["nc"]


def kernel(inputs: np.ndarray, W: np.ndarray, _trace: bool = False):
    """inputs: (512, 64, 1024) f32; W: (1, 1024, 1024) f32.
    Returns (64, 16, 64) f32."""
    nc = _get_nc()
    consts = _host_consts()
    w0 = W[0].astype(np.float16)
    wh = np.ascontiguousarray(w0.reshape(HC, 128, 1024).transpose(1, 0, 2))
    wth = np.ascontiguousarray(w0.reshape(1024, OC, 128).transpose(2, 1, 0))
    xf = inputs.astype(np.float16)              # (512, 64, 1024)
    in_maps = []
    for c in range(N_CORES):
        xs = xf[:, c * BL:(c + 1) * BL, :]      # (512, BL, 1024)
        xh = np.ascontiguousarray(
            xs.reshape(SC, 128, BL, 1024).transpose(1, 2, 0, 3))
        xth = np.ascontiguousarray(
            xs.reshape(512, BL, HC, 128).transpose(3, 1, 2, 0))
        m = {"xh": xh, "xth": xth, "wh": wh, "wth": wth}
        m.update(consts)
        in_maps.append(m)
    kw = {}
    if _trace:
        kw = dict(trace=True, trace_cores=list(range(N_CORES)),
                  stitch_traces=False)
    res = run_bass_kernel_spmd(nc, in_maps, core_ids=list(range(N_CORES)),
                               **kw)
    out = np.concatenate([res.results[c]["out"] for c in range(N_CORES)],
                         axis=0)
    if _trace:
        return out.astype(np.float32), res
    return out.astype(np.float32)


# revision 19
# speedup vs baseline: 1.1523x; 1.1523x over previous
"""Trainium2 Bass kernel for capsule dynamic routing (nn_Capsule).

Reference (per batch item b):
    u = x_b @ W; logits = 0
    for i in 4:
        c = softmax(logits, axis=capsule)
        t_j = sum_s c[s,j] * u[s, j*64:(j+1)*64]; v = squash(t)
        if i < 3: logits[s,j] += u[s, jblk] . v_j

Never materializes u. By linearity:
    y_q   = sum_s c[s,q] x_s        (GEMM over S, X natural layout)
    T     = y @ W, t = blockdiag(T) (dense q=(b,j) partition layout)
    P     = W^T' Vblk               (Vblk = block-diag of v)
    upd   = X P                     (GEMM over H, X^T layout)

Sharding: batch-parallel, 8 batch items per core, W replicated.
Dense partition index q = b*16 + j (8 batch x 16 capsules = 128).

Perf design (vs. the first working version):
  - All big operands are pre-packed on HOST into the exact f16 SBUF
    layouts (x, x^T, W, W^T) -> contiguous HWDGE DMA loads, no SWDGE
    casts, no element-strided on-chip transpose DMAs.
  - T and P are computed in the dense q layout: 16 matmuls of
    N=512/128 per step instead of 64 padded-256 matmuls.
  - squash runs in natural layout (q on partitions, d on free axis):
    DVE reduce + ACT sqrt, no PE broadcast matmuls.

HW lessons kept from the previous version:
  - ScalarE activation(Copy) for every PSUM f32 -> f16 cast (DVE dies).
  - Each PE-transpose output gets its own PSUM tile.
  - matmul start=True lazily zeroes the PSUM bank for the out AP's
    partitions; partition-disjoint groups interleave with
    skip_group_check=True.
  - f16 constants come from host DRAM, not memset.
"""
import numpy as np
from contextlib import ExitStack

import concourse.bass as bass
import concourse.bacc as bacc
import concourse.tile as tile
from concourse import mybir
from concourse.bass_utils import run_bass_kernel_spmd

f16 = mybir.dt.float16
f32 = mybir.dt.float32
COPY = mybir.ActivationFunctionType.Copy
EXP = mybir.ActivationFunctionType.Exp

S, B, H = 512, 64, 1024
NCAP, DCAP = 16, 64
ROUTINGS = 4
N_CORES = 8
BL = B // N_CORES          # 8 batch items per core
SC = S // 128              # 4 s-chunks
HC = H // 128              # 8 h-chunks
OC = H // 128              # 8 o-chunks (o = NCAP*DCAP = 1024)


def _act_copy(nc, out, in_):
    nc.scalar.activation(out=out, in_=in_, func=COPY, scale=1.0, alpha=0.0)


def _build_kernel(tc, out_d, x_d, xt_d, w_d, wt_d, id16_d, id32_d, eps_d,
                  cpad_d, logits_d, vblk_d, tt_d):
    nc = tc.nc
    ctx = ExitStack()
    const = ctx.enter_context(tc.tile_pool(name="const", bufs=1))
    work = ctx.enter_context(tc.tile_pool(name="work", bufs=2))
    small = ctx.enter_context(tc.tile_pool(name="small", bufs=2))
    # PSUM budget: 8 banks. big: y_ps/T_ps/PT_ps/u_ps cycle one 4-bank
    # slot (bufs=1); tp: all transposes, 4 x 1 bank.
    ps_big = ctx.enter_context(tc.tile_pool(name="ps_big", bufs=1,
                                            space="PSUM"))
    ps_tp = ctx.enter_context(tc.tile_pool(name="ps_tp", bufs=4, space="PSUM"))

    # ---------- persistent tensors ----------
    x16 = const.tile([128, BL, SC, 1024], f16)    # X natural (s_loc, b, sc, h)
    xt16 = const.tile([128, BL, HC, 512], f16)    # X^T (h_loc, b, hc, s)
    w16 = const.tile([128, HC, 1024], f16)        # W natural (h_loc, hc, o)
    wt16 = const.tile([128, OC, 1024], f16)       # W^T (o_loc, oc, h)
    id16 = const.tile([128, 128], f16)            # eye(128) f16
    id32 = const.tile([128, 128], f32)            # eye(128) f32
    eps = const.tile([128, 1], f32)
    cpad = const.tile([128, BL, SC, 32], f16)     # c, cols 16-31 zero pad
    logits = const.tile([128, BL, SC, 16], f32)
    vblk = const.tile([128, OC, 128], f16)        # block-diag v, dense q cols
    tt16 = const.tile([128, 128], f16)            # t^T split, zeros elsewhere

    # ---------- loads (all contiguous, host-prepacked f16) ----------
    # Small consts FIRST so iteration-0 compute can ride the x16 chunks.
    nc.sync.dma_start(out=id16[:], in_=id16_d[:])
    nc.sync.dma_start(out=id32[:], in_=id32_d[:])
    nc.sync.dma_start(out=eps[:], in_=eps_d[:])
    nc.sync.dma_start(out=cpad[:], in_=cpad_d[:])
    nc.sync.dma_start(out=logits[:], in_=logits_d[:])
    nc.sync.dma_start(out=vblk[:], in_=vblk_d[:])
    nc.sync.dma_start(out=tt16[:], in_=tt_d[:])
    for sc in range(SC):
        for b in range(BL):
            nc.sync.dma_start(out=x16[:, b, sc, :], in_=x_d[:, b, sc, :])
    nc.sync.dma_start(out=w16[:], in_=w_d[:])
    nc.sync.dma_start(out=wt16[:], in_=wt_d[:])
    for b in range(BL):
        nc.sync.dma_start(out=xt16[:, b, :, :], in_=xt_d[:, b, :, :])

    v32 = None
    for it in range(ROUTINGS):
        last = it == ROUTINGS - 1

        # ---------- y = C^T X: (2g x 4b x 32pad part, 1024 h) f32 ----------
        y_ps = ps_big.tile([128, 2, 1024], f32, tag="big", name=f"y_ps{it}")
        for g in range(2):
            for half in range(2):
                for sc in range(SC):
                    for b_ in range(4):
                        b = 4 * g + b_
                        nc.tensor.matmul(
                            y_ps[32 * b_:32 * b_ + 32, g,
                                 512 * half:512 * half + 512],
                            cpad[:, b, sc, :],
                            x16[:, b, sc, 512 * half:512 * half + 512],
                            start=(sc == 0), stop=(sc == SC - 1),
                            skip_group_check=True,
                            tile_position=(0, 32 * b_))
        y_sb = work.tile([128, 2, 1024], f16, tag="y_sb")
        for g in range(2):
            for half in range(2):
                _act_copy(nc, y_sb[:, g, 512 * half:512 * half + 512],
                          y_ps[:, g, 512 * half:512 * half + 512])

        # ---------- y^T via PE transposes, dense-packed q cols ----------
        yt = work.tile([128, HC, 128], f16, tag="yt")
        for hc in range(HC):
            for g in range(2):
                tp = ps_tp.tile([128, 128], f16, tag="tp",
                                name=f"yt_tp{it}_{hc}_{g}")
                nc.tensor.matmul(
                    tp[:], y_sb[:, g, 128 * hc:128 * hc + 128], id16[:],
                    is_transpose=True, skip_group_check=True)
                # dense capsule-major pack: yt col q = j*8 + b
                src = tp.rearrange("p (b j) -> p j b", j=32)[:, 0:16, :]
                dst = yt[:, hc, :].rearrange(
                    "p (j b) -> p j b", b=8)[:, :, 4 * g:4 * g + 4]
                if (hc + 2 * g) % 2 == 0:
                    nc.vector.tensor_copy(dst, src)
                else:
                    _act_copy(nc, dst, src)

        # ---------- T = y W: (128 q, 1024 o) f32, dense ----------
        t_ps = ps_big.tile([128, 1024], f32, tag="big", name=f"t_ps{it}")
        for half in range(2):
            for hc in range(HC):
                nc.tensor.matmul(
                    t_ps[:, 512 * half:512 * half + 512],
                    yt[:, hc, :],
                    w16[:, hc, 512 * half:512 * half + 512],
                    start=(hc == 0), stop=(hc == HC - 1))
        t_sb16 = work.tile([128, 1024], f16, tag="t_sb16")
        for half in range(2):
            _act_copy(nc, t_sb16[:, 512 * half:512 * half + 512],
                      t_ps[:, 512 * half:512 * half + 512])

        # ---------- T^T chunks via PE transpose; aligned extracts -------
        # tt16[par*64 + d, q] = t[q, d] for q in [8j, 8j+8), j = 2oc+par;
        # other cells stay zero (host-initialized).
        for oc in range(OC):
            ttp = ps_tp.tile([128, 128], f16, tag="tp",
                             name=f"tt_tp{it}_{oc}")
            nc.tensor.matmul(ttp[:], t_sb16[:, 128 * oc:128 * oc + 128],
                             id16[:], is_transpose=True,
                             skip_group_check=True)
            for par in range(2):
                j = 2 * oc + par
                p0 = 64 * par
                nc.vector.tensor_copy(
                    tt16[p0:p0 + 64, 8 * j:8 * j + 8],
                    ttp[p0:p0 + 64, 8 * j:8 * j + 8])

        # ---------- transpose to t[q, d'] (d' = d + 64*(j%2)) ----------
        ttq = ps_tp.tile([128, 128], f16, tag="tp", name=f"ttq{it}")
        nc.tensor.matmul(ttq[:], tt16[:], id16[:],
                         is_transpose=True, skip_group_check=True)
        t_sb = small.tile([128, 128], f16, tag="t_sb")
        nc.vector.tensor_copy(t_sb[:], ttq[:])

        # ---------- squash: v = t / sqrt(sum_d t^2 + eps) ----------
        t2 = small.tile([128, 128], f32, tag="t2")
        nc.vector.tensor_mul(t2[:], t_sb[:], t_sb[:])
        ssum = small.tile([128, 1], f32, tag="ssum")
        nc.vector.reduce_sum(ssum[:], t2[:], axis=mybir.AxisListType.X)
        # rsqrt on DVE: quake initial guess + 2 Newton steps (keeps the
        # Scalar engine on a single act table: copy/exp only)
        nn_ = small.tile([128, 1], f32, tag="nn_")
        nc.vector.tensor_scalar_add(nn_[:], ssum[:], 1e-7)
        sh = small.tile([128, 1], mybir.dt.int32, tag="sh")
        nc.vector.tensor_scalar(
            out=sh[:], in0=nn_.bitcast(mybir.dt.int32), scalar1=1,
            scalar2=None, op0=mybir.AluOpType.arith_shift_right)
        r0i = small.tile([128, 1], mybir.dt.int32, tag="r0i")
        nc.vector.tensor_scalar(
            out=r0i[:], in0=sh[:], scalar1=-1, scalar2=0x5F3759DF,
            op0=mybir.AluOpType.mult, op1=mybir.AluOpType.add)
        rs = small.tile([128, 1], f32, tag="rs")
        rprev = r0i.bitcast(f32)
        for newt in range(2):
            ra = small.tile([128, 1], f32, tag=f"ra{newt}")
            nc.vector.tensor_mul(ra[:], rprev, rprev)
            rb = small.tile([128, 1], f32, tag=f"rb{newt}")
            nc.vector.tensor_mul(rb[:], ra[:], nn_[:])
            rc = small.tile([128, 1], f32, tag=f"rc{newt}")
            nc.vector.tensor_scalar(
                out=rc[:], in0=rb[:], scalar1=-0.5, scalar2=1.5,
                op0=mybir.AluOpType.mult, op1=mybir.AluOpType.add)
            rd = rs if newt == 1 else small.tile([128, 1], f32,
                                                 tag=f"rd{newt}")
            nc.vector.tensor_mul(rd[:], rprev, rc[:])
            rprev = rd[:]
        if last:
            v32 = small.tile([128, 128], f32, tag="v32")
            nc.vector.tensor_mul(v32[:], t_sb[:],
                                 rs.broadcast_to([128, 128]))
            break
        v16 = small.tile([128, 128], f16, tag="v16")
        nc.vector.tensor_mul(v16[:], t_sb[:], rs.broadcast_to([128, 128]))

        # ---------- scatter v into block-diag vblk (via transpose) -------
        vtp = ps_tp.tile([128, 128], f16, tag="tp", name=f"v_tp{it}")
        nc.tensor.matmul(vtp[:], v16[:], id16[:],
                         is_transpose=True, skip_group_check=True)
        for oc in range(OC):
            for par in range(2):
                j = 2 * oc + par
                p0, p1 = 64 * par, 64 * par + 64
                nc.vector.tensor_copy(vblk[p0:p1, oc, 8 * j:8 * j + 8],
                                      vtp[p0:p1, 8 * j:8 * j + 8])

        # ---------- P^T = Vblk^T W^T: (128 q, 1024 h) f32, dense -------
        pt_ps = ps_big.tile([128, 1024], f32, tag="big", name=f"pt_ps{it}")
        for half in range(2):
            for oc in range(OC):
                nc.tensor.matmul(
                    pt_ps[:, 512 * half:512 * half + 512],
                    vblk[:, oc, :],
                    wt16[:, oc, 512 * half:512 * half + 512],
                    start=(oc == 0), stop=(oc == OC - 1))
        pt_sb = work.tile([128, 1024], f16, tag="pt_sb")
        for half in range(2):
            _act_copy(nc, pt_sb[:, 512 * half:512 * half + 512],
                      pt_ps[:, 512 * half:512 * half + 512])

        # ---------- transpose P^T -> P natural, batch-major cols --------
        p_sb = work.tile([128, HC, 128], f16, tag="p_sb")
        for hc in range(HC):
            ptp = ps_tp.tile([128, 128], f16, tag="tp",
                             name=f"p_tp{it}_{hc}")
            nc.tensor.matmul(
                ptp[:], pt_sb[:, 128 * hc:128 * hc + 128], id16[:],
                is_transpose=True, skip_group_check=True)
            # permute cols capsule-major q=j*8+b -> batch-major b*16+j
            nc.vector.tensor_copy(
                p_sb[:, hc, :].rearrange("p (b j) -> p j b", j=16),
                ptp.rearrange("p (j b) -> p j b", b=8))

        # ---------- update = X P via X^T (dense 16-col weights) ----------
        u_ps = ps_big.tile([128, 2, 512], f32, tag="big", name=f"u_ps{it}")
        for g in range(2):
            for hc in range(HC):
                for b_ in range(4):
                    b = 4 * g + b_
                    nc.tensor.matmul(
                        u_ps[32 * b_:32 * b_ + 16, g, :],
                        p_sb[:, hc, 16 * b:16 * b + 16],
                        xt16[:, b, hc, :],
                        start=(hc == 0), stop=(hc == HC - 1),
                        skip_group_check=True,
                        tile_position=(0, 32 * b_))
        u_sb = work.tile([128, 2, 512], f32, tag="u_sb")
        for g in range(2):
            nc.vector.tensor_copy(u_sb[:, g, :], u_ps[:, g, :])

        # ---------- transpose update, accumulate logits ----------
        for sc in range(SC):
            for g in range(2):
                utp = ps_tp.tile([128, 128], f32, tag="tp",
                                 name=f"ut_tp{it}_{sc}_{g}")
                nc.tensor.matmul(
                    utp[:], u_sb[:, g, 128 * sc:128 * sc + 128], id32[:],
                    is_transpose=True, skip_group_check=True)
                src = utp.rearrange("p (b j) -> p b j", j=32)
                nc.vector.tensor_add(
                    logits[:, 4 * g:4 * g + 4, sc, :],
                    logits[:, 4 * g:4 * g + 4, sc, :], src[:, :, 0:16])

        # ---------- softmax over capsules -> cpad ----------
        for sc in range(SC):
            ex = small.tile([128, BL, 16], f32, tag="ex")
            nc.scalar.activation(out=ex[:], in_=logits[:, :, sc, :],
                                 func=EXP, scale=1.0, alpha=0.0)
            sm = small.tile([128, BL, 1], f32, tag="sm")
            nc.vector.reduce_sum(sm[:], ex[:], axis=mybir.AxisListType.X)
            rc = small.tile([128, BL, 1], f32, tag="rc")
            nc.vector.reciprocal(rc[:], sm[:])
            nc.vector.tensor_mul(cpad[:, :, sc, 0:16], ex[:],
                                 rc.broadcast_to([128, BL, 16]))

    # ---------- out[b, j, d] = v32[q = j*8 + b, d + 64*(j%2)] ----------
    for j in range(NCAP):
        out_ap = bass.AP(tensor=out_d.tensor, offset=64 * j,
                         ap=[[1024, 8], [1, 64]])
        c0 = 64 * (j % 2)
        nc.sync.dma_start(out=out_ap,
                          in_=v32[8 * j:8 * j + 8, c0:c0 + 64])
    ctx.close()


_CACHE = {}


def _host_consts():
    ident = np.ascontiguousarray(np.eye(128, dtype=np.float16))
    ident32 = np.ascontiguousarray(np.eye(128, dtype=np.float32))
    eps = np.full((128, 1), 1e-7, np.float32)
    cpad = np.zeros((128, BL, SC, 32), np.float16)
    cpad[:, :, :, 0:16] = 1.0 / NCAP          # iteration-0 softmax is exact
    logi = np.zeros((128, BL, SC, 16), np.float32)
    vblk = np.zeros((128, OC, 128), np.float16)
    tti = np.zeros((128, 128), np.float16)
    return {"id16": ident, "id32": ident32, "epsb": eps,
            "cpadi": cpad, "logi": logi, "vblki": vblk, "tti": tti}


def _get_nc():
    if "nc" not in _CACHE:
        nc = bacc.Bacc("TRN2", target_bir_lowering=False, debug=False)
        x_d = nc.dram_tensor("xh", [128, BL, SC, 1024], f16,
                             kind="ExternalInput")
        xt_d = nc.dram_tensor("xth", [128, BL, HC, 512], f16,
                              kind="ExternalInput")
        w_d = nc.dram_tensor("wh", [128, HC, 1024], f16,
                             kind="ExternalInput")
        wt_d = nc.dram_tensor("wth", [128, OC, 1024], f16,
                              kind="ExternalInput")
        id16_d = nc.dram_tensor("id16", [128, 128], f16,
                                kind="ExternalInput")
        id32_d = nc.dram_tensor("id32", [128, 128], f32,
                                kind="ExternalInput")
        eps_d = nc.dram_tensor("epsb", [128, 1], f32, kind="ExternalInput")
        cpad_d = nc.dram_tensor("cpadi", [128, BL, SC, 32], f16,
                                kind="ExternalInput")
        logits_d = nc.dram_tensor("logi", [128, BL, SC, 16], f32,
                                  kind="ExternalInput")
        vblk_d = nc.dram_tensor("vblki", [128, OC, 128], f16,
                                kind="ExternalInput")
        tt_d = nc.dram_tensor("tti", [128, 128], f16, kind="ExternalInput")
        out_d = nc.dram_tensor("out", [BL, NCAP, DCAP], f32,
                               kind="ExternalOutput")
        with tile.TileContext(nc) as tc:
            _build_kernel(tc, out_d.ap(), x_d.ap(), xt_d.ap(), w_d.ap(),
                          wt_d.ap(), id16_d.ap(), id32_d.ap(), eps_d.ap(),
                          cpad_d.ap(), logits_d.ap(), vblk_d.ap(), tt_d.ap())
        nc.compile()
        _CACHE["nc"] = nc
    return _CACHE["nc"]


def kernel(inputs: np.ndarray, W: np.ndarray, _trace: bool = False):
    """inputs: (512, 64, 1024) f32; W: (1, 1024, 1024) f32.
    Returns (64, 16, 64) f32."""
    nc = _get_nc()
    consts = _host_consts()
    w0 = W[0].astype(np.float16)
    wh = np.ascontiguousarray(w0.reshape(HC, 128, 1024).transpose(1, 0, 2))
    wth = np.ascontiguousarray(w0.reshape(1024, OC, 128).transpose(2, 1, 0))
    xf = inputs.astype(np.float16)              # (512, 64, 1024)
    in_maps = []
    for c in range(N_CORES):
        xs = xf[:, c * BL:(c + 1) * BL, :]      # (512, BL, 1024)
        xh = np.ascontiguousarray(
            xs.reshape(SC, 128, BL, 1024).transpose(1, 2, 0, 3))
        xth = np.ascontiguousarray(
            xs.reshape(512, BL, HC, 128).transpose(3, 1, 2, 0))
        m = {"xh": xh, "xth": xth, "wh": wh, "wth": wth}
        m.update(consts)
        in_maps.append(m)
    kw = {}
    if _trace:
        kw = dict(trace=True, trace_cores=list(range(N_CORES)),
                  stitch_traces=False)
    res = run_bass_kernel_spmd(nc, in_maps, core_ids=list(range(N_CORES)),
                               **kw)
    out = np.concatenate([res.results[c]["out"] for c in range(N_CORES)],
                         axis=0)
    if _trace:
        return out.astype(np.float32), res
    return out.astype(np.float32)


# revision 22
# speedup vs baseline: 1.1611x; 1.0076x over previous
"""Trainium2 Bass kernel for capsule dynamic routing (nn_Capsule).

Reference (per batch item b):
    u = x_b @ W; logits = 0
    for i in 4:
        c = softmax(logits, axis=capsule)
        t_j = sum_s c[s,j] * u[s, j*64:(j+1)*64]; v = squash(t)
        if i < 3: logits[s,j] += u[s, jblk] . v_j

Never materializes u. By linearity (dense q = j*8 + b, 16 caps x 8 batch
= 128 partitions):
    y_q    = sum_s c[s,q] x_s          (GEMM over S, X natural layout)
    T      = y @ W  (dense q x 1024)   -> t = blockdiag(T), UNNORMALIZED,
             scattered straight into Vblk (identical layout)
    P^T    = Vblk^T W^T, then scaled by rs[q] = rsqrt(|t_q|^2 + eps)
             during the PSUM->SBUF copy (squash deferred off the PE path)
    upd    = X P via X^T
The squash norm is computed from Vblk^2 with a PE partition-reduce that
overlaps the P^T matmuls; rsqrt is a quake-style bit trick + 2 Newton
steps on DVE (Scalar stays on one act table: copy/exp only).

HW lessons encoded:
  - ScalarE activation(Copy) for every PSUM f32 -> f16 cast (DVE dies).
  - Each PE-transpose output gets its own PSUM tile.
  - Engine APs need 32-aligned partition bases on PSUM; DMA cannot
    touch PSUM; tensor_tensor_reduce faults the device.
  - matmul start=True lazily zeroes the PSUM bank for the out AP's
    partitions; partition-disjoint groups use skip_group_check=True.
  - f16 constants come from host DRAM, not memset.
  - Per-queue DMA bandwidth is ~1/16 of the core's share: split every
    big tensor into chunks and order chunks by first use.
"""
import numpy as np
from contextlib import ExitStack

import concourse.bass as bass
import concourse.bacc as bacc
import concourse.tile as tile
from concourse import mybir
from concourse.bass_utils import run_bass_kernel_spmd

f16 = mybir.dt.float16
f32 = mybir.dt.float32
i32 = mybir.dt.int32
COPY = mybir.ActivationFunctionType.Copy
EXP = mybir.ActivationFunctionType.Exp

S, B, H = 512, 64, 1024
NCAP, DCAP = 16, 64
ROUTINGS = 4
N_CORES = 8
BL = B // N_CORES          # 8 batch items per core
SC = S // 128              # 4 s-chunks
HC = H // 128              # 8 h-chunks
OC = H // 128              # 8 o-chunks (o = NCAP*DCAP = 1024)


def _act_copy(nc, out, in_, scale=1.0):
    nc.scalar.activation(out=out, in_=in_, func=COPY, scale=scale, alpha=0.0)


def _quake_rsqrt(nc, small, n_ap, eps_val, shape, it):
    """rs = 1/sqrt(n + eps) on DVE (bit trick + 2 Newton steps).
    n_ap may live in PSUM (f32). Returns an SBUF f32 AP of `shape`."""
    nn_ = small.tile(shape, f32, tag="qk_nn", name=f"nn{it}")
    nc.vector.tensor_scalar_add(nn_[:], n_ap, eps_val)
    sh = small.tile(shape, i32, tag="qk_sh", name=f"sh{it}")
    nc.vector.tensor_scalar(
        out=sh[:], in0=nn_.bitcast(i32), scalar1=1, scalar2=None,
        op0=mybir.AluOpType.arith_shift_right)
    r0i = small.tile(shape, i32, tag="qk_r0", name=f"r0i{it}")
    nc.vector.tensor_scalar(
        out=r0i[:], in0=sh[:], scalar1=-1, scalar2=0x5F3759DF,
        op0=mybir.AluOpType.mult, op1=mybir.AluOpType.add)
    rprev = r0i.bitcast(f32)
    for newt in range(2):
        ra = small.tile(shape, f32, tag="qk_ra", name=f"ra{it}{newt}")
        nc.vector.tensor_mul(ra[:], rprev, rprev)
        rb = small.tile(shape, f32, tag="qk_rb", name=f"rb{it}{newt}")
        nc.vector.tensor_mul(rb[:], ra[:], nn_[:])
        rc = small.tile(shape, f32, tag="qk_rc", name=f"rc{it}{newt}")
        nc.vector.tensor_scalar(
            out=rc[:], in0=rb[:], scalar1=-0.5, scalar2=1.5,
            op0=mybir.AluOpType.mult, op1=mybir.AluOpType.add)
        rd = small.tile(shape, f32, tag="qk_rd", name=f"rd{it}{newt}")
        nc.vector.tensor_mul(rd[:], rprev, rc[:])
        rprev = rd[:]
    return rprev


def _build_kernel(tc, out_d, x_d, xt_d, w_d, id16_d, id32_d,
                  cpad_d, logits_d, vblk_d, tt_d, ones1_d, onescol_d):
    nc = tc.nc
    ctx = ExitStack()
    const = ctx.enter_context(tc.tile_pool(name="const", bufs=1))
    work = ctx.enter_context(tc.tile_pool(name="work", bufs=1))
    small = ctx.enter_context(tc.tile_pool(name="small", bufs=2))
    # PSUM budget: 8 banks. big: y_ps/T_ps/PT_ps/u_ps cycle one 4-bank
    # slot (bufs=1); tp: all transposes + tiny norm tiles, 4 x 1 bank.
    ps_big = ctx.enter_context(tc.tile_pool(name="ps_big", bufs=1,
                                            space="PSUM"))
    ps_tp = ctx.enter_context(tc.tile_pool(name="ps_tp", bufs=4, space="PSUM"))

    # ---------- persistent tensors ----------
    x16 = const.tile([128, BL, SC, 1024], f16)    # X natural (s_loc, b, sc, h)
    xt16 = const.tile([128, BL, HC, 512], f16)    # X^T (h_loc, b, hc, s)
    w16 = const.tile([128, HC, 1024], f16)        # W natural (h_loc, hc, o)
    wt16 = const.tile([128, OC, 1024], f16)       # W^T (derived on-chip)
    id16 = const.tile([128, 128], f16)            # eye(128) f16
    id32 = const.tile([128, 128], f32)            # eye(128) f32
    ones1 = const.tile([1, 1], f32)
    onescol = const.tile([128, 1], f16)
    cpad = const.tile([128, BL, SC, 32], f16)     # c, cols 16-31 zero pad
    logits = const.tile([128, BL, SC, 16], f32)
    vblk = const.tile([128, OC, 128], f16)        # block-diag t, dense q cols
    tt16 = const.tile([128, 128], f16)            # t^T split (last iter only)

    # ---------- loads: consts first, then chunks by first use ----------
    nc.sync.dma_start(out=id16[:], in_=id16_d[:])
    nc.sync.dma_start(out=id32[:], in_=id32_d[:])
    nc.sync.dma_start(out=ones1[:], in_=ones1_d[:])
    nc.sync.dma_start(out=onescol[:], in_=onescol_d[:])
    nc.sync.dma_start(out=cpad[:], in_=cpad_d[:])
    nc.sync.dma_start(out=logits[:], in_=logits_d[:])
    nc.sync.dma_start(out=vblk[:], in_=vblk_d[:])
    nc.sync.dma_start(out=tt16[:], in_=tt_d[:])
    for hc in range(HC):
        nc.sync.dma_start(out=w16[:, hc, :], in_=w_d[:, hc, :])
    for sc in range(SC):
        for half in range(2):
            for b in range(BL):
                sl = slice(512 * half, 512 * half + 512)
                nc.sync.dma_start(out=x16[:, b, sc, sl],
                                  in_=x_d[:, b, sc, sl])
    for b in range(BL):
        nc.sync.dma_start(out=xt16[:, b, :, :], in_=xt_d[:, b, :, :])

    # ---------- derive W^T on-chip (PE idle during the x/xt load) -----
    for hc in range(HC):
        for oc in range(OC):
            tpw = ps_tp.tile([128, 128], f16, tag="tp",
                             name=f"w_tp{hc}_{oc}")
            nc.tensor.matmul(tpw[:], w16[:, hc, 128 * oc:128 * oc + 128],
                             id16[:], is_transpose=True,
                             skip_group_check=True)
            nc.vector.tensor_copy(wt16[:, oc, 128 * hc:128 * hc + 128],
                                  tpw[:])

    v32 = None
    for it in range(ROUTINGS):
        last = it == ROUTINGS - 1

        # ---------- y = C^T X: (2g x 4b x 32pad part, 1024 h) f32 --------
        y_ps = ps_big.tile([128, 2, 1024], f32, tag="big", name=f"y_ps{it}")
        for g in range(2):
            for sc in range(SC):
                for b_ in range(4):
                    b = 4 * g + b_
                    for half in range(2):   # same weights both halves
                        nc.tensor.matmul(
                            y_ps[32 * b_:32 * b_ + 32, g,
                                 512 * half:512 * half + 512],
                            cpad[:, b, sc, :],
                            x16[:, b, sc, 512 * half:512 * half + 512],
                            start=(sc == 0), stop=(sc == SC - 1),
                            skip_group_check=True,
                            tile_position=(0, 32 * b_))
        y_sb = work.tile([128, 2, 1024], f16, tag="y_sb")
        for g in range(2):
            for half in range(2):
                _act_copy(nc, y_sb[:, g, 512 * half:512 * half + 512],
                          y_ps[:, g, 512 * half:512 * half + 512])

        # ---------- y^T via PE transposes, dense capsule-major pack ------
        yt = work.tile([128, HC, 128], f16, tag="yt")
        for hc in range(HC):
            for g in range(2):
                tp = ps_tp.tile([128, 128], f16, tag="tp",
                                name=f"yt_tp{it}_{hc}_{g}")
                nc.tensor.matmul(
                    tp[:], y_sb[:, g, 128 * hc:128 * hc + 128], id16[:],
                    is_transpose=True, skip_group_check=True)
                src = tp.rearrange("p (b j) -> p j b", j=32)[:, 0:16, :]
                dst = yt[:, hc, :].rearrange(
                    "p (j b) -> p j b", b=8)[:, :, 4 * g:4 * g + 4]
                nc.vector.tensor_copy(dst, src)

        # ---------- T = y W: (128 q, 1024 o) f32, dense ----------
        t_ps = ps_big.tile([128, 1024], f32, tag="big", name=f"t_ps{it}")
        for half in range(2):
            for hc in range(HC):
                nc.tensor.matmul(
                    t_ps[:, 512 * half:512 * half + 512],
                    yt[:, hc, :],
                    w16[:, hc, 512 * half:512 * half + 512],
                    start=(hc == 0), stop=(hc == HC - 1))
        t_sb16 = work.tile([128, 1024], f16, tag="t_sb16")
        for half in range(2):
            _act_copy(nc, t_sb16[:, 512 * half:512 * half + 512],
                      t_ps[:, 512 * half:512 * half + 512])

        # ---------- T^T chunks via PE transpose; extract diag blocks -----
        # Unnormalized t goes straight into vblk (same layout); the last
        # iteration assembles tt16 instead (needs t in q-major for out).
        for oc in range(OC):
            ttp = ps_tp.tile([128, 128], f16, tag="tp",
                             name=f"tt_tp{it}_{oc}")
            nc.tensor.matmul(ttp[:], t_sb16[:, 128 * oc:128 * oc + 128],
                             id16[:], is_transpose=True,
                             skip_group_check=True)
            for par in range(2):
                j = 2 * oc + par
                p0 = 64 * par
                if last:
                    nc.vector.tensor_copy(
                        tt16[p0:p0 + 64, 8 * j:8 * j + 8],
                        ttp[p0:p0 + 64, 8 * j:8 * j + 8])
                else:
                    nc.vector.tensor_copy(
                        vblk[p0:p0 + 64, oc, 8 * j:8 * j + 8],
                        ttp[p0:p0 + 64, 8 * j:8 * j + 8])

        if last:
            # ---------- t[q, d'] via transpose; squash; output ----------
            ttq = ps_tp.tile([128, 128], f16, tag="tp", name=f"ttq{it}")
            nc.tensor.matmul(ttq[:], tt16[:], id16[:],
                             is_transpose=True, skip_group_check=True)
            t_sb = small.tile([128, 128], f16, tag="t_sb")
            nc.vector.tensor_copy(t_sb[:], ttq[:])
            t2 = small.tile([128, 128], f32, tag="t2")
            nc.vector.tensor_mul(t2[:], t_sb[:], t_sb[:])
            ssum = small.tile([128, 1], f32, tag="ssum")
            nc.vector.reduce_sum(ssum[:], t2[:], axis=mybir.AxisListType.X)
            rs = _quake_rsqrt(nc, small, ssum[:], 1e-7, [128, 1], it)
            v32 = small.tile([128, 128], f32, tag="v32")
            nc.vector.tensor_mul(v32[:], t_sb[:],
                                 rs.broadcast_to([128, 128]))
            break

        # ---------- norm from vblk^2 (overlaps P^T matmuls) ----------
        sq = work.tile([128, OC, 128], f16, tag="sq")
        for oc in range(OC):
            nc.vector.tensor_mul(sq[:, oc, :], vblk[:, oc, :],
                                 vblk[:, oc, :])

        # ---------- P^T = Vblk^T W^T with norm reduce interleaved -------
        pt_ps = ps_big.tile([128, 1024], f32, tag="big", name=f"pt_ps{it}")
        n_ps = ps_tp.tile([1, 128], f32, tag="tp", name=f"n_ps{it}")
        for oc in range(OC):
            nc.tensor.matmul(
                pt_ps[:, 0:512], vblk[:, oc, :], wt16[:, oc, 0:512],
                start=(oc == 0), stop=(oc == OC - 1))
        for oc in range(OC):
            nc.tensor.matmul(
                n_ps[:], onescol[:], sq[:, oc, :],
                start=(oc == 0), stop=(oc == OC - 1))
        for oc in range(OC):
            nc.tensor.matmul(
                pt_ps[:, 512:1024], vblk[:, oc, :], wt16[:, oc, 512:1024],
                start=(oc == 0), stop=(oc == OC - 1))
        rs_row = _quake_rsqrt(nc, small, n_ps[:], 1e-7, [1, 128], it)
        # broadcast rs_row (1,128) -> column (128,1) via 1-contraction MM
        rs_ps = ps_tp.tile([128, 1], f32, tag="tp", name=f"rs_ps{it}")
        nc.tensor.matmul(rs_ps[:], rs_row, ones1[:],
                         skip_group_check=True)
        rs_col = small.tile([128, 1], f32, tag="rs_col")
        nc.vector.tensor_copy(rs_col[:], rs_ps[:])

        # ---------- pt_sb = rs[q] * P^T  (squash applied here) ----------
        pt_sb = work.tile([128, 1024], f16, tag="pt_sb")
        for half in range(2):
            nc.scalar.activation(
                out=pt_sb[:, 512 * half:512 * half + 512],
                in_=pt_ps[:, 512 * half:512 * half + 512],
                func=COPY, scale=rs_col[:], alpha=0.0)

        # ---------- transpose P^T -> P natural (capsule-major cols) -----
        p_sb = work.tile([128, HC, 128], f16, tag="p_sb")
        for hc in range(HC):
            ptp = ps_tp.tile([128, 128], f16, tag="tp",
                             name=f"p_tp{it}_{hc}")
            nc.tensor.matmul(
                ptp[:], pt_sb[:, 128 * hc:128 * hc + 128], id16[:],
                is_transpose=True, skip_group_check=True)
            nc.vector.tensor_copy(p_sb[:, hc, :], ptp[:])

        # ---------- update = X P via X^T (strided 16-col weights) -------
        u_ps = ps_big.tile([128, 2, 512], f32, tag="big", name=f"u_ps{it}")
        for g in range(2):
            for hc in range(HC):
                for b_ in range(4):
                    b = 4 * g + b_
                    wcols = p_sb[:, hc, :].rearrange(
                        "p (j b) -> p b j", b=8)[:, b, :]
                    nc.tensor.matmul(
                        u_ps[32 * b_:32 * b_ + 16, g, :],
                        wcols,
                        xt16[:, b, hc, :],
                        start=(hc == 0), stop=(hc == HC - 1),
                        skip_group_check=True,
                        tile_position=(0, 32 * b_))
        u_sb = work.tile([128, 2, 512], f32, tag="u_sb")
        for g in range(2):
            nc.vector.tensor_copy(u_sb[:, g, :], u_ps[:, g, :])

        # ---------- transpose update, accumulate logits ----------
        for sc in range(SC):
            for g in range(2):
                utp = ps_tp.tile([128, 128], f32, tag="tp",
                                 name=f"ut_tp{it}_{sc}_{g}")
                nc.tensor.matmul(
                    utp[:], u_sb[:, g, 128 * sc:128 * sc + 128], id32[:],
                    is_transpose=True, skip_group_check=True)
                src = utp.rearrange("p (b j) -> p b j", j=32)
                nc.vector.tensor_add(
                    logits[:, 4 * g:4 * g + 4, sc, :],
                    logits[:, 4 * g:4 * g + 4, sc, :], src[:, :, 0:16])

        # ---------- softmax over capsules -> cpad ----------
        for sc in range(SC):
            ex = small.tile([128, BL, 16], f32, tag="ex")
            nc.scalar.activation(out=ex[:], in_=logits[:, :, sc, :],
                                 func=EXP, scale=1.0, alpha=0.0)
            sm = small.tile([128, BL, 1], f32, tag="sm")
            nc.vector.reduce_sum(sm[:], ex[:], axis=mybir.AxisListType.X)
            rc = small.tile([128, BL, 1], f32, tag="rc")
            nc.vector.reciprocal(rc[:], sm[:])
            nc.vector.tensor_mul(cpad[:, :, sc, 0:16], ex[:],
                                 rc.broadcast_to([128, BL, 16]))

    # ---------- out[b, j, d] = v32[q = j*8 + b, d + 64*(j%2)] ----------
    for j in range(NCAP):
        out_ap = bass.AP(tensor=out_d.tensor, offset=64 * j,
                         ap=[[1024, 8], [1, 64]])
        c0 = 64 * (j % 2)
        nc.sync.dma_start(out=out_ap,
                          in_=v32[8 * j:8 * j + 8, c0:c0 + 64])
    ctx.close()


_CACHE = {}


def _host_consts():
    ident = np.ascontiguousarray(np.eye(128, dtype=np.float16))
    ident32 = np.ascontiguousarray(np.eye(128, dtype=np.float32))
    ones1 = np.ones((1, 1), np.float32)
    onescol = np.ones((128, 1), np.float16)
    cpad = np.zeros((128, BL, SC, 32), np.float16)
    cpad[:, :, :, 0:16] = 1.0 / NCAP          # iteration-0 softmax is exact
    logi = np.zeros((128, BL, SC, 16), np.float32)
    vblk = np.zeros((128, OC, 128), np.float16)
    tti = np.zeros((128, 128), np.float16)
    return {"id16": ident, "id32": ident32, "ones1": ones1,
            "onescol": onescol, "cpadi": cpad, "logi": logi,
            "vblki": vblk, "tti": tti}


def _get_nc():
    if "nc" not in _CACHE:
        nc = bacc.Bacc("TRN2", target_bir_lowering=False, debug=False)
        x_d = nc.dram_tensor("xh", [128, BL, SC, 1024], f16,
                             kind="ExternalInput")
        xt_d = nc.dram_tensor("xth", [128, BL, HC, 512], f16,
                              kind="ExternalInput")
        w_d = nc.dram_tensor("wh", [128, HC, 1024], f16,
                             kind="ExternalInput")
        id16_d = nc.dram_tensor("id16", [128, 128], f16,
                                kind="ExternalInput")
        id32_d = nc.dram_tensor("id32", [128, 128], f32,
                                kind="ExternalInput")
        ones1_d = nc.dram_tensor("ones1", [1, 1], f32, kind="ExternalInput")
        onescol_d = nc.dram_tensor("onescol", [128, 1], f16,
                                   kind="ExternalInput")
        cpad_d = nc.dram_tensor("cpadi", [128, BL, SC, 32], f16,
                                kind="ExternalInput")
        logits_d = nc.dram_tensor("logi", [128, BL, SC, 16], f32,
                                  kind="ExternalInput")
        vblk_d = nc.dram_tensor("vblki", [128, OC, 128], f16,
                                kind="ExternalInput")
        tt_d = nc.dram_tensor("tti", [128, 128], f16, kind="ExternalInput")
        out_d = nc.dram_tensor("out", [BL, NCAP, DCAP], f32,
                               kind="ExternalOutput")
        with tile.TileContext(nc) as tc:
            _build_kernel(tc, out_d.ap(), x_d.ap(), xt_d.ap(), w_d.ap(),
                          id16_d.ap(), id32_d.ap(), cpad_d.ap(),
                          logits_d.ap(), vblk_d.ap(), tt_d.ap(),
                          ones1_d.ap(), onescol_d.ap())
        nc.compile()
        _CACHE["nc"] = nc
    return _CACHE["nc"]


def kernel(inputs: np.ndarray, W: np.ndarray, _trace: bool = False):
    """inputs: (512, 64, 1024) f32; W: (1, 1024, 1024) f32.
    Returns (64, 16, 64) f32."""
    nc = _get_nc()
    consts = _host_consts()
    w0 = W[0].astype(np.float16)
    wh = np.ascontiguousarray(w0.reshape(HC, 128, 1024).transpose(1, 0, 2))
    xf = inputs.astype(np.float16)              # (512, 64, 1024)
    in_maps = []
    for c in range(N_CORES):
        xs = xf[:, c * BL:(c + 1) * BL, :]      # (512, BL, 1024)
        xh = np.ascontiguousarray(
            xs.reshape(SC, 128, BL, 1024).transpose(1, 2, 0, 3))
        xth = np.ascontiguousarray(
            xs.reshape(512, BL, HC, 128).transpose(3, 1, 2, 0))
        m = {"xh": xh, "xth": xth, "wh": wh}
        m.update(consts)
        in_maps.append(m)
    kw = {}
    if _trace:
        kw = dict(trace=True, trace_cores=list(range(N_CORES)),
                  stitch_traces=False)
    res = run_bass_kernel_spmd(nc, in_maps, core_ids=list(range(N_CORES)),
                               **kw)
    out = np.concatenate([res.results[c]["out"] for c in range(N_CORES)],
                         axis=0)
    if _trace:
        return out.astype(np.float32), res
    return out.astype(np.float32)
